# revision 1
# baseline (speedup 1.0000x reference)
"""Trainium2 Bass kernel for nn_CustomMSELoss (penalty-weighted MSE - variance).

loss = mean(penalty * (y_true - y_pred)^2) - var(y_pred, ddof=1)
  penalty = 6 where y_true < percentile(y_true, 15)
          = 6 where y_true > percentile(y_true, 85)
          = 1 otherwise

Strategy (8 NeuronCores, data-parallel over the element axis):
  Each core streams its 1/8 shard of (y_true, y_pred) once from HBM via the
  two HWDGE DMA queues (y_true on SP/sync, y_pred on ACT/scalar) and
  computes, fully fused per 128x2048 tile:
    - ACT: |y_true|;  r^2 (+ per-partition accumulate);  y_pred^2 (+ acc)
    - DVE: r = y_true - y_pred;  sum(r^2 * [|y_true|<=T_MID]) via stt (+ acc)
    - PE : sum(y_pred) as ones^T @ y_pred accumulated in PSUM
  Engine busy times (per 4M-elem shard): DVE ~68us, ACT ~82us, PE ~23us,
  DMA ~94us (roofline) -- the stream is DMA-bound, compute fully hidden.

  The exact percentiles are order statistics. The host counts elements
  beyond +-T_OUT (np.count_nonzero, exact in f32), ranks the order
  statistic inside the narrow value band (T_IN..T_OUT, ~1% of elements)
  and applies the exact r^2 correction for elements between the fixed
  device threshold T_MID and the true percentile thresholds. All
  arithmetic that must match the device (subtract, square, abs, compares)
  is replayed in float32. If the band does not contain the percentile
  ranks (pathological input distribution), falls back to an exact host
  computation.
"""

import os
import sys

import numpy as np

# ---------------------------------------------------------------- constants
N_TOTAL = 33554432
NCORES = 8
SHARD = N_TOTAL // NCORES          # 4_194_304
P = 128                            # SBUF partitions
MM_N = 512                         # matmul free-dim chunk

LEFT_PCT = 15.0
RIGHT_PCT = 85.0
PENALTY = 6.0
VAR_W = 1.0

# Fixed value-band thresholds around the expected +-1.0364 percentiles of
# N(0,1).  T_MID is the on-device penalty-mask boundary; the host corrects
# exactly within the (T_IN, T_OUT) band, which must contain T_MID and both
# true percentile values.
T_MID = np.float32(1.04)
T_IN = np.float32(1.025)
T_OUT = np.float32(1.055)

_CONCOURSE_PATHS = ["/opt/trn_rl_repo", "/root/.axon_site/_ro/trn_rl_repo"]


def _import_concourse():
    try:
        import concourse.bass  # noqa: F401
    except ImportError:
        for p in _CONCOURSE_PATHS:
            if os.path.isdir(p) and p not in sys.path:
                sys.path.insert(0, p)
        import concourse.bass  # noqa: F401


# ---------------------------------------------------------------- device IR
_NC_CACHE = {}

BEST_CFG = dict(f=2048, dma_span=1, io_bufs=4, mid_bufs=3,
                dma_yt="sync", dma_yp="sync")


def build_nc(f=2048, dma_span=1, io_bufs=4, mid_bufs=3,
             dma_yt="sync", dma_yp="scalar", repeat=1):
    """Build the per-core Bass program (identical on all cores).

    repeat>1 re-runs the whole streaming pass in a hardware For_i loop (for
    HW timing via wall-clock deltas); outputs stay valid since accumulator
    slots are overwritten and writeback happens after the loop.
    """
    _import_concourse()
    from contextlib import ExitStack

    import concourse.bacc as bacc
    import concourse.tile as tile
    from concourse import mybir

    ntiles = SHARD // (P * f)
    assert SHARD == P * f * ntiles and ntiles % dma_span == 0

    key = (f, dma_span, io_bufs, mid_bufs, dma_yt, dma_yp, repeat)
    if key in _NC_CACHE:
        return _NC_CACHE[key]

    fp32 = mybir.dt.float32
    Alu = mybir.AluOpType
    Act = mybir.ActivationFunctionType

    nc = bacc.Bacc()
    yt_d = nc.declare_dram_parameter("y_true", [SHARD], fp32, isOutput=False)
    yp_d = nc.declare_dram_parameter("y_pred", [SHARD], fp32, isOutput=False)
    # acc layout along free dim: [r2 | yp2 | smid] x ntiles
    out_acc = nc.declare_dram_parameter("acc", [P, 3 * ntiles], fp32,
                                        isOutput=True)
    out_yps = nc.declare_dram_parameter("ypsum", [1, MM_N], fp32, isOutput=True)

    fd = f * dma_span
    ytv = yt_d[:].rearrange("(n p f) -> n p f", p=P, f=fd)
    ypv = yp_d[:].rearrange("(n p f) -> n p f", p=P, f=fd)

    def dma_eng(which, idx):
        name = {"yt": dma_yt, "yp": dma_yp}[which]
        if name == "alt":
            order = ["sync", "scalar"] if which == "yt" else ["scalar", "sync"]
            name = order[idx % 2]
        return getattr(nc, name)

    with ExitStack() as ctx:
        tc = ctx.enter_context(tile.TileContext(nc))
        accp = ctx.enter_context(tc.tile_pool(name="accp", bufs=1))
        psp = ctx.enter_context(tc.tile_pool(name="psum", bufs=1, space="PSUM"))

        # engine-private accumulators (separate tiles -> no cross-engine
        # false dependencies from tile-granular dependency tracking)
        acc_act = accp.tile([P, 2 * ntiles], fp32)   # r2 | yp2
        acc_dve = accp.tile([P, ntiles], fp32)       # smid
        scr_dve = accp.tile([P, f], fp32)
        scr_act = accp.tile([P, f], fp32)
        ones = accp.tile([P, 1], fp32)
        yps_sb = accp.tile([1, MM_N], fp32)
        nc.vector.memset(ones, 1.0)
        ps = psp.tile([1, MM_N], fp32)

        loop = ExitStack()
        with loop:
            if repeat > 1:
                loop.enter_context(tc.For_i(0, repeat, 1))
            io = loop.enter_context(tc.tile_pool(name="io", bufs=io_bufs))
            mid = loop.enter_context(tc.tile_pool(name="mid", bufs=mid_bufs))

            for td in range(ntiles // dma_span):
                ytd = io.tile([P, fd], fp32, tag="yt")
                ypd = io.tile([P, fd], fp32, tag="yp")
                dma_eng("yt", td).dma_start(out=ytd, in_=ytv[td])
                dma_eng("yp", td).dma_start(out=ypd, in_=ypv[td])
                for ts_i in range(dma_span):
                    t = td * dma_span + ts_i
                    yt = ytd[:, ts_i * f: (ts_i + 1) * f]
                    yp = ypd[:, ts_i * f: (ts_i + 1) * f]

                    r = mid.tile([P, f], fp32, tag="r")
                    nc.vector.tensor_sub(r, yt, yp)

                    ayt = mid.tile([P, f], fp32, tag="ayt")
                    r2 = mid.tile([P, f], fp32, tag="r2")
                    nc.scalar.activation(ayt, yt, Act.Abs)
                    nc.scalar.activation(r2, r, Act.Square,
                                         accum_out=acc_act[:, t:t + 1])
                    nc.scalar.activation(
                        scr_act, yp, Act.Square,
                        accum_out=acc_act[:, ntiles + t:ntiles + t + 1])

                    # (|y_t| <= T_MID) * r^2  summed per partition
                    nc.vector.scalar_tensor_tensor(
                        scr_dve, ayt, float(T_MID), r2, Alu.is_le, Alu.mult,
                        accum_out=acc_dve[:, t:t + 1])

                    # sum(y_pred) on the otherwise-idle PE: ones^T @ yp
                    # chunks accumulated in a single PSUM region
                    n_mm = f // MM_N
                    for c in range(n_mm):
                        nc.tensor.matmul(
                            ps[:, :], ones, yp[:, c * MM_N:(c + 1) * MM_N],
                            start=(t == 0 and c == 0),
                            stop=(t == ntiles - 1 and c == n_mm - 1))

        # writeback (outside the repeat loop)
        nc.vector.tensor_copy(yps_sb, ps)
        nc.sync.dma_start(out=out_acc[:, 0:2 * ntiles], in_=acc_act)
        nc.sync.dma_start(out=out_acc[:, 2 * ntiles:3 * ntiles], in_=acc_dve)
        nc.sync.dma_start(out=out_yps[:, :], in_=yps_sb)

    nc.finalize()
    _NC_CACHE[key] = nc
    return nc


# ---------------------------------------------------------------- device run
def run_device(y_pred, y_true, trace=False):
    """Shard across 8 cores, run the Bass kernel, return per-core outputs."""
    _import_concourse()
    from concourse.bass_utils import run_bass_kernel_spmd

    nc = build_nc(**BEST_CFG)
    in_maps = []
    for i in range(NCORES):
        sl = slice(i * SHARD, (i + 1) * SHARD)
        in_maps.append(
            {
                "y_true": np.ascontiguousarray(y_true[sl]),
                "y_pred": np.ascontiguousarray(y_pred[sl]),
            }
        )
    res = run_bass_kernel_spmd(nc, in_maps, list(range(NCORES)), trace=trace)
    return res


def _combine(results):
    """Combine per-core device partials (float64)."""
    acc = np.stack([np.asarray(r["acc"], dtype=np.float64) for r in results])
    nt = acc.shape[-1] // 3
    s_r2 = acc[:, :, 0 * nt: 1 * nt].sum()
    s_yp2 = acc[:, :, 1 * nt: 2 * nt].sum()
    s_mid = acc[:, :, 2 * nt: 3 * nt].sum()
    s_yp = np.stack([np.asarray(r["ypsum"], dtype=np.float64)
                     for r in results]).sum()
    return s_r2, s_yp2, s_mid, s_yp


# ------------------------------------------------------------- host finishing
def _f32_percentile_pos(n, pct):
    """Replicate jnp.percentile's float32 position arithmetic."""
    q = np.float32(np.float64(pct) / 100.0)
    nf = np.float32(n)
    pos = np.float32(q * np.float32(nf - np.float32(1.0)))
    low = np.floor(pos)
    high = np.ceil(pos)
    hw = np.float32(pos - low)
    lw = np.float32(np.float32(1.0) - hw)
    low = int(min(max(low, 0.0), float(n - 1)))
    high = int(min(max(high, 0.0), float(n - 1)))
    return low, high, lw, hw


def _fallback_numpy(y_pred, y_true):
    """Exact host computation (used only if the value band misses)."""
    y_pred = y_pred.astype(np.float32)
    y_true = y_true.astype(np.float32)
    n = y_true.size
    vs = np.sort(y_true)

    def pctl(pct):
        low, high, lw, hw = _f32_percentile_pos(n, pct)
        return np.float32(
            np.float32(vs[low] * lw) + np.float32(vs[high] * hw)
        )

    lo_t = pctl(LEFT_PCT)
    hi_t = pctl(RIGHT_PCT)
    r = (y_true - y_pred).astype(np.float32)
    r2 = (r * r).astype(np.float64)
    pen = np.where((y_true < lo_t) | (y_true > hi_t), PENALTY, 1.0)
    mse = (pen * r2).mean()
    var = y_pred.astype(np.float64).var(ddof=1)
    return np.float32(mse - VAR_W * var)


def _order_stat_threshold(win_sorted, base_rank, n, pct):
    """Exact percentile from a sorted value-band slice.

    win_sorted holds (ascending) all elements with global ranks
    [base_rank, base_rank + len(win_sorted)).  Returns None if the
    percentile's order statistics are not inside the window.
    """
    low, high, lw, hw = _f32_percentile_pos(n, pct)
    i_lo = low - base_rank
    i_hi = high - base_rank
    if i_lo < 0 or i_hi < 0 or i_hi >= win_sorted.size or i_lo >= win_sorted.size:
        return None
    lv = win_sorted[i_lo]
    hv = win_sorted[i_hi]
    return np.float32(np.float32(lv * lw) + np.float32(hv * hw))


def kernel(y_pred, y_true):
    y_pred = np.asarray(y_pred, dtype=np.float32).reshape(-1)
    y_true = np.asarray(y_true, dtype=np.float32).reshape(-1)
    assert y_pred.shape == (N_TOTAL,) and y_true.shape == (N_TOTAL,)

    res = run_device(y_pred, y_true)
    s_r2, s_yp2, s_mid, s_yp = _combine(res.results)

    n = float(N_TOTAL)
    # exact global ranks of the band edges (host-side, integer-exact)
    c_l = int(np.count_nonzero(y_true < -T_OUT))
    c_r = int(np.count_nonzero(y_true > T_OUT))

    # value bands around the two percentiles (host-side ranking, o(N) output)
    band_l = np.sort(y_true[(y_true >= -T_OUT) & (y_true <= -T_IN)])
    band_r = np.sort(y_true[(y_true >= T_IN) & (y_true <= T_OUT)])

    lo_t = _order_stat_threshold(band_l, c_l, N_TOTAL, LEFT_PCT)
    base_r = N_TOTAL - c_r - band_r.size
    hi_t = _order_stat_threshold(band_r, base_r, N_TOTAL, RIGHT_PCT)

    if (
        lo_t is None
        or hi_t is None
        or not (-float(T_OUT) < lo_t < -float(T_IN))
        or not (float(T_IN) < hi_t < float(T_OUT))
    ):
        return _fallback_numpy(y_pred, y_true)

    # exact correction over the bands: device penalized |y|>T_MID, we want
    # y<lo_t or y>hi_t.  All disagreeing elements lie inside the bands.
    sel = ((y_true >= -T_OUT) & (y_true <= -T_IN)) | (
        (y_true >= T_IN) & (y_true <= T_OUT)
    )
    yb = y_true[sel]
    rb = (yb - y_pred[sel]).astype(np.float32)
    r2b = (rb * rb).astype(np.float64)
    want = (yb < lo_t) | (yb > hi_t)
    dev = np.abs(yb) > T_MID
    corr = (r2b * (want.astype(np.float64) - dev.astype(np.float64))).sum()

    tails = (s_r2 - s_mid) + corr
    mse = (s_r2 + (PENALTY - 1.0) * tails) / n
    var = (s_yp2 - (s_yp * s_yp) / n) / (n - 1.0)
    return np.float32(mse - VAR_W * var)


if __name__ == "__main__":
    rng = np.random.default_rng(0)
    yp = rng.standard_normal(N_TOTAL, dtype=np.float32)
    yt = rng.standard_normal(N_TOTAL, dtype=np.float32)
    print(kernel(yp, yt))



# revision 2
# speedup vs baseline: 1.4813x; 1.4813x over previous
"""Trainium2 Bass kernel for nn_CustomMSELoss (penalty-weighted MSE - var).

loss = mean(penalty * (y_true - y_pred)^2) - var(y_pred, ddof=1)
  penalty = 6 outside the [15th, 85th] percentiles of y_true, else 1.

Device strategy (8 NeuronCores, data-parallel, fp16 streaming):
  The rel-err budget (2e-2) admits 16-bit input streaming: the host casts
  both tensors to fp16 and each core streams its 1/8 shard (2 x 8 MiB)
  once from HBM -- half the f32 bytes, so the DMA roofline halves to
  ~51 us/core.  All percentile/threshold logic stays exact on the host
  (f32), and every device fp16 decision (the |y_true|<=T_MID mask) is
  replayed bit-exactly on the host, so quantization only perturbs the
  bulk sums by ~1e-5 relative.

  Per 128x4096 fp16 tile, engines split so each stays under the DMA
  roofline (ScalarE has no 16-bit speedup; DVE gets 2x/4x perf modes):
    DVE: r = yt - yp (2x);  smid = sum((|yt|<=T)*r^2) via stt (+acc)
    ACT: |yt|;  r^2 (+acc Sr2)
    PE : G += yp_chunk^T @ yp_chunk  (trace(G) = sum(yp^2));
         ones^T @ yp  (sum(yp))
  Stages are software-pipelined with skews (s1=1, s2=2) so no engine
  waits on another engine's current tile.

  Host finishing: exact order statistics inside a narrow value band
  around the +-1.0364 expected percentiles (counts + sort of ~1% of
  elements), then an exact correction of the penalty mass between the
  fixed device threshold T_MID and the true thresholds, replaying the
  device's fp16 arithmetic.  Falls back to exact f32 numpy if the band
  misses (pathological inputs).
"""

import os
import sys

import numpy as np

# ---------------------------------------------------------------- constants
N_TOTAL = 33554432
NCORES = 8
SHARD = N_TOTAL // NCORES          # 4_194_304
P = 128                            # SBUF partitions
MM_N = 512                         # ones-matmul free-dim chunk

LEFT_PCT = 15.0
RIGHT_PCT = 85.0
PENALTY = 6.0
VAR_W = 1.0

# Fixed value-band around the expected +-1.0364 percentiles of N(0,1).
# T_MID is the on-device mask boundary; the host corrects exactly within
# (T_IN, T_OUT), which must contain T_MID and both true thresholds.
T_MID = np.float32(1.04)
T_IN = np.float32(1.025)
T_OUT = np.float32(1.055)

DTYPE = "float16"                  # device streaming dtype

_CONCOURSE_PATHS = ["/opt/trn_rl_repo", "/root/.axon_site/_ro/trn_rl_repo"]


def _import_concourse():
    try:
        import concourse.bass  # noqa: F401
    except ImportError:
        for p in _CONCOURSE_PATHS:
            if os.path.isdir(p) and p not in sys.path:
                sys.path.insert(0, p)
        import concourse.bass  # noqa: F401


# ---------------------------------------------------------------- device IR
_NC_CACHE = {}

BEST_CFG = dict(arch="safe", f=4096, io_bufs=4, mid_bufs=4, s1=1, s2=2)


def build_nc(arch="safe", f=4096, io_bufs=4, mid_bufs=4, s1=1, s2=2,
             repeat=1):
    """Per-core Bass program (identical on all cores).

    repeat>1 re-runs the streaming pass in a hardware For_i loop (used
    only for timing via wall-clock deltas); accumulator columns are
    overwritten per iteration and written back after the loop.
    """
    _import_concourse()
    from contextlib import ExitStack

    import concourse.bacc as bacc
    import concourse.tile as tile
    from concourse import mybir

    ntiles = SHARD // (P * f)
    assert SHARD == P * f * ntiles and 0 <= s1 < s2

    key = (arch, f, io_bufs, mid_bufs, s1, s2, repeat)
    if key in _NC_CACHE:
        return _NC_CACHE[key]

    fp32 = mybir.dt.float32
    f16 = getattr(mybir.dt, DTYPE)
    Alu = mybir.AluOpType
    Act = mybir.ActivationFunctionType

    nc = bacc.Bacc()
    yt_d = nc.declare_dram_parameter("y_true", [SHARD], f16, isOutput=False)
    yp_d = nc.declare_dram_parameter("y_pred", [SHARD], f16, isOutput=False)
    # acc columns: [Sr2 | Smid] x ntiles (per-partition fp32 sums)
    out_acc = nc.declare_dram_parameter("acc", [P, 2 * ntiles], fp32,
                                        isOutput=True)
    out_g = nc.declare_dram_parameter("gram", [P, P], fp32, isOutput=True)
    out_yps = nc.declare_dram_parameter("ypsum", [1, MM_N], fp32,
                                        isOutput=True)

    ytv = yt_d[:].rearrange("(n p f) -> n p f", p=P, f=f)
    ypv = yp_d[:].rearrange("(n p f) -> n p f", p=P, f=f)

    with ExitStack() as ctx:
        tc = ctx.enter_context(tile.TileContext(nc))
        accp = ctx.enter_context(tc.tile_pool(name="accp", bufs=1))
        psp = ctx.enter_context(tc.tile_pool(name="psum", bufs=1,
                                             space="PSUM"))

        acc_a = accp.tile([P, 2 * ntiles], fp32)
        scr = accp.tile([P, f], f16)
        ones = accp.tile([P, 1], f16)
        yps_sb = accp.tile([1, MM_N], fp32)
        g_sb = accp.tile([P, P], fp32)
        nc.vector.memset(ones, 1.0)
        ps = psp.tile([1, MM_N], fp32)
        gp = psp.tile([P, P], fp32)

        n_mm = f // MM_N
        n_gr = f // P

        loop = ExitStack()
        with loop:
            if repeat > 1:
                loop.enter_context(tc.For_i(0, repeat, 1))
            io = loop.enter_context(tc.tile_pool(name="io", bufs=io_bufs))
            mid = loop.enter_context(tc.tile_pool(name="mid", bufs=mid_bufs))

            yts, yps, rs, ms, r2s = {}, {}, {}, {}, {}

            for t in range(ntiles + s2):
                if t < ntiles:
                    yts[t] = io.tile([P, f], f16, tag="yt", name=f"yt{t}")
                    yps[t] = io.tile([P, f], f16, tag="yp", name=f"yp{t}")
                    nc.sync.dma_start(out=yts[t], in_=ytv[t])
                    nc.sync.dma_start(out=yps[t], in_=ypv[t])
                    yt, yp = yts[t], yps[t]
                    rs[t] = mid.tile([P, f], f16, tag="r", name=f"r{t}")
                    nc.vector.tensor_sub(rs[t], yt, yp)
                    ms[t] = mid.tile([P, f], f16, tag="m", name=f"m{t}")
                    if arch == "v2":
                        nc.vector.tensor_scalar(
                            ms[t], yt, 0.0, float(T_MID),
                            Alu.abs_max, Alu.is_le)
                    elif arch == "ts":
                        # |yt| on DVE (tensor_scalar abs_max, 4x-eligible)
                        nc.vector.tensor_scalar(
                            ms[t], yt, 0.0, None, Alu.abs_max)
                    else:
                        nc.scalar.activation(ms[t], yt, Act.Abs)
                    for c in range(n_gr):
                        nc.tensor.matmul(
                            gp[:, :], yp[:, c * P:(c + 1) * P],
                            yp[:, c * P:(c + 1) * P],
                            start=(t == 0 and c == 0),
                            stop=(t == ntiles - 1 and c == n_gr - 1))
                    for c in range(n_mm):
                        nc.tensor.matmul(
                            ps[:, :], ones, yp[:, c * MM_N:(c + 1) * MM_N],
                            start=(t == 0 and c == 0),
                            stop=(t == ntiles - 1 and c == n_mm - 1))
                u = t - s1
                if 0 <= u < ntiles:
                    r2s[u] = mid.tile([P, f], f16, tag="r2", name=f"r2_{u}")
                    nc.scalar.activation(r2s[u], rs[u], Act.Square,
                                         accum_out=acc_a[:, u:u + 1])
                    del rs[u]
                u = t - s2
                if u >= 0:
                    col = acc_a[:, ntiles + u:ntiles + u + 1]
                    if arch == "v2":
                        nc.vector.tensor_tensor_reduce(
                            scr, r2s[u], ms[u], 1.0, 0.0,
                            Alu.mult, Alu.add, accum_out=col)
                    else:
                        nc.vector.scalar_tensor_tensor(
                            scr, ms[u], float(T_MID), r2s[u],
                            Alu.is_le, Alu.mult, accum_out=col)
                    del ms[u], r2s[u]

        nc.vector.tensor_copy(yps_sb, ps)
        nc.vector.tensor_copy(g_sb, gp)
        nc.sync.dma_start(out=out_acc[:, :], in_=acc_a)
        nc.sync.dma_start(out=out_g[:, :], in_=g_sb)
        nc.sync.dma_start(out=out_yps[:, :], in_=yps_sb)

    nc.finalize()
    _NC_CACHE[key] = nc
    return nc


# ---------------------------------------------------------------- device run
def make_in_maps(y_pred, y_true):
    """Cast to the streaming dtype and shard across cores."""
    yp16 = np.ascontiguousarray(y_pred, dtype=DTYPE)
    yt16 = np.ascontiguousarray(y_true, dtype=DTYPE)
    in_maps = []
    for i in range(NCORES):
        sl = slice(i * SHARD, (i + 1) * SHARD)
        in_maps.append({"y_true": yt16[sl], "y_pred": yp16[sl]})
    return in_maps


def run_device(y_pred, y_true, trace=False):
    _import_concourse()
    from concourse.bass_utils import run_bass_kernel_spmd

    nc = build_nc(**BEST_CFG)
    in_maps = make_in_maps(y_pred, y_true)
    return run_bass_kernel_spmd(nc, in_maps, list(range(NCORES)),
                                trace=trace)


def _combine(results):
    """Combine per-core device partials (float64)."""
    acc = np.stack([np.asarray(r["acc"], dtype=np.float64) for r in results])
    nt = acc.shape[-1] // 2
    s_r2 = acc[:, :, 0:nt].sum()
    s_mid = acc[:, :, nt:2 * nt].sum()
    s_yp2 = sum(
        np.diagonal(np.asarray(r["gram"], dtype=np.float64)).sum()
        for r in results
    )
    s_yp = np.stack([np.asarray(r["ypsum"], dtype=np.float64)
                     for r in results]).sum()
    return s_r2, s_yp2, s_mid, s_yp


# ------------------------------------------------------------- host finishing
def _f32_percentile_pos(n, pct):
    """Replicate jnp.percentile's float32 position arithmetic."""
    q = np.float32(np.float64(pct) / 100.0)
    nf = np.float32(n)
    pos = np.float32(q * np.float32(nf - np.float32(1.0)))
    low = np.floor(pos)
    high = np.ceil(pos)
    hw = np.float32(pos - low)
    lw = np.float32(np.float32(1.0) - hw)
    low = int(min(max(low, 0.0), float(n - 1)))
    high = int(min(max(high, 0.0), float(n - 1)))
    return low, high, lw, hw


def _fallback_numpy(y_pred, y_true):
    """Exact host computation (used only if the value band misses)."""
    y_pred = y_pred.astype(np.float32)
    y_true = y_true.astype(np.float32)
    n = y_true.size
    vs = np.sort(y_true)

    def pctl(pct):
        low, high, lw, hw = _f32_percentile_pos(n, pct)
        return np.float32(
            np.float32(vs[low] * lw) + np.float32(vs[high] * hw)
        )

    lo_t = pctl(LEFT_PCT)
    hi_t = pctl(RIGHT_PCT)
    r = (y_true - y_pred).astype(np.float32)
    r2 = (r * r).astype(np.float64)
    pen = np.where((y_true < lo_t) | (y_true > hi_t), PENALTY, 1.0)
    mse = (pen * r2).mean()
    var = y_pred.astype(np.float64).var(ddof=1)
    return np.float32(mse - VAR_W * var)


def _order_stat_threshold(win_sorted, base_rank, n, pct):
    """Exact percentile from a sorted value-band slice (or None)."""
    low, high, lw, hw = _f32_percentile_pos(n, pct)
    i_lo = low - base_rank
    i_hi = high - base_rank
    if (i_lo < 0 or i_hi < 0 or i_hi >= win_sorted.size
            or i_lo >= win_sorted.size):
        return None
    lv = win_sorted[i_lo]
    hv = win_sorted[i_hi]
    return np.float32(np.float32(lv * lw) + np.float32(hv * hw))


def kernel(y_pred, y_true):
    y_pred = np.asarray(y_pred, dtype=np.float32).reshape(-1)
    y_true = np.asarray(y_true, dtype=np.float32).reshape(-1)
    assert y_pred.shape == (N_TOTAL,) and y_true.shape == (N_TOTAL,)

    res = run_device(y_pred, y_true)
    s_r2, s_yp2, s_mid, s_yp = _combine(res.results)

    n = float(N_TOTAL)
    # exact global ranks of the band edges (host-side, integer-exact)
    c_l = int(np.count_nonzero(y_true < -T_OUT))
    c_r = int(np.count_nonzero(y_true > T_OUT))

    band_l = np.sort(y_true[(y_true >= -T_OUT) & (y_true <= -T_IN)])
    band_r = np.sort(y_true[(y_true >= T_IN) & (y_true <= T_OUT)])

    lo_t = _order_stat_threshold(band_l, c_l, N_TOTAL, LEFT_PCT)
    base_r = N_TOTAL - c_r - band_r.size
    hi_t = _order_stat_threshold(band_r, base_r, N_TOTAL, RIGHT_PCT)

    if (
        lo_t is None
        or hi_t is None
        or not (-float(T_OUT) < lo_t < -float(T_IN))
        or not (float(T_IN) < hi_t < float(T_OUT))
    ):
        return _fallback_numpy(y_pred, y_true)

    # exact correction over the bands: device penalized |fp16(y)| > T_MID,
    # we want y < lo_t or y > hi_t.  fp16 rounding moves values by
    # <= 2^-11 * 1.04 ~ 5e-4, so every disagreeing element lies inside
    # the (T_IN, T_OUT) bands.  Replay the device's fp16 arithmetic.
    sel = ((y_true >= -T_OUT) & (y_true <= -T_IN)) | (
        (y_true >= T_IN) & (y_true <= T_OUT)
    )
    yb = y_true[sel]
    yb16 = yb.astype(DTYPE)
    pb16 = y_pred[sel].astype(DTYPE)
    rb = (yb16.astype(np.float32) - pb16.astype(np.float32)).astype(DTYPE)
    r2b = (rb.astype(np.float32) ** 2).astype(DTYPE).astype(np.float64)
    want = (yb < lo_t) | (yb > hi_t)
    dev = np.abs(yb16.astype(np.float32)) > np.float32(T_MID)
    corr = (r2b * (want.astype(np.float64) - dev.astype(np.float64))).sum()

    tails = (s_r2 - s_mid) + corr
    mse = (s_r2 + (PENALTY - 1.0) * tails) / n
    var = (s_yp2 - (s_yp * s_yp) / n) / (n - 1.0)
    return np.float32(mse - VAR_W * var)


if __name__ == "__main__":
    rng = np.random.default_rng(0)
    yp = rng.standard_normal(N_TOTAL, dtype=np.float32)
    yt = rng.standard_normal(N_TOTAL, dtype=np.float32)
    print(kernel(yp, yt))


# revision 5
# speedup vs baseline: 1.4849x; 1.0024x over previous
"""Trainium2 Bass kernel for nn_CustomMSELoss (penalty-weighted MSE - var).

loss = mean(penalty * (y_true - y_pred)^2) - var(y_pred, ddof=1)
  penalty = 6 outside the [15th, 85th] percentiles of y_true, else 1.

Device strategy (8 NeuronCores, data-parallel, fp16 streaming):
  The rel-err budget (2e-2) admits 16-bit input streaming: the host casts
  both tensors to fp16 and each core streams its 1/8 shard (2 x 8 MiB)
  once from HBM -- half the f32 bytes, so the DMA roofline halves to
  ~51 us/core.  All percentile/threshold logic stays exact on the host
  (f32), and every device fp16 decision (the |y_true|<=T_MID mask) is
  replayed bit-exactly on the host, so quantization only perturbs the
  bulk sums by ~1e-5 relative.

  Per 128x4096 fp16 tile, engines split so each stays under the DMA
  roofline (ScalarE has no 16-bit speedup; DVE gets 2x/4x perf modes):
    DVE: r = yt - yp (2x);  smid = sum((|yt|<=T)*r^2) via stt (+acc)
    ACT: |yt|;  r^2 (+acc Sr2)
    PE : G += yp_chunk^T @ yp_chunk  (trace(G) = sum(yp^2));
         ones^T @ yp  (sum(yp))
  Stages are software-pipelined with skews (s1=1, s2=2) so no engine
  waits on another engine's current tile.

  Host finishing: exact order statistics inside a narrow value band
  around the +-1.0364 expected percentiles (counts + sort of ~1% of
  elements), then an exact correction of the penalty mass between the
  fixed device threshold T_MID and the true thresholds, replaying the
  device's fp16 arithmetic.  Falls back to exact f32 numpy if the band
  misses (pathological inputs).
"""

import os
import sys

import numpy as np

# ---------------------------------------------------------------- constants
N_TOTAL = 33554432
NCORES = 8
SHARD = N_TOTAL // NCORES          # 4_194_304
P = 128                            # SBUF partitions
MM_N = 512                         # ones-matmul free-dim chunk

LEFT_PCT = 15.0
RIGHT_PCT = 85.0
PENALTY = 6.0
VAR_W = 1.0

# Fixed value-band around the expected +-1.0364 percentiles of N(0,1).
# T_MID is the on-device mask boundary; the host corrects exactly within
# (T_IN, T_OUT), which must contain T_MID and both true thresholds.
T_MID = np.float32(1.04)
T_IN = np.float32(1.025)
T_OUT = np.float32(1.055)

DTYPE = "float16"                  # device streaming dtype

_CONCOURSE_PATHS = ["/opt/trn_rl_repo", "/root/.axon_site/_ro/trn_rl_repo"]


def _import_concourse():
    try:
        import concourse.bass  # noqa: F401
    except ImportError:
        for p in _CONCOURSE_PATHS:
            if os.path.isdir(p) and p not in sys.path:
                sys.path.insert(0, p)
        import concourse.bass  # noqa: F401


# ---------------------------------------------------------------- device IR
_NC_CACHE = {}

BEST_CFG = dict(arch="safe", f=4096, io_bufs=4, mid_bufs=4, s1=1, s2=2)


def build_nc(arch="safe", f=4096, io_bufs=4, mid_bufs=4, s1=1, s2=2,
             repeat=1):
    """Per-core Bass program (identical on all cores).

    repeat>1 re-runs the streaming pass in a hardware For_i loop (used
    only for timing via wall-clock deltas); accumulator columns are
    overwritten per iteration and written back after the loop.
    """
    _import_concourse()
    from contextlib import ExitStack

    import concourse.bacc as bacc
    import concourse.tile as tile
    from concourse import mybir

    ntiles = SHARD // (P * f)
    assert SHARD == P * f * ntiles and 0 <= s1 < s2

    key = (arch, f, io_bufs, mid_bufs, s1, s2, repeat)
    if key in _NC_CACHE:
        return _NC_CACHE[key]

    fp32 = mybir.dt.float32
    f16 = getattr(mybir.dt, DTYPE)
    Alu = mybir.AluOpType
    Act = mybir.ActivationFunctionType

    nc = bacc.Bacc()
    yt_d = nc.declare_dram_parameter("y_true", [SHARD], f16, isOutput=False)
    yp_d = nc.declare_dram_parameter("y_pred", [SHARD], f16, isOutput=False)
    # acc columns: [Sr2 | Smid] x ntiles (per-partition fp32 sums)
    out_acc = nc.declare_dram_parameter("acc", [P, 2 * ntiles], fp32,
                                        isOutput=True)
    out_g = nc.declare_dram_parameter("gram", [P, P], fp32, isOutput=True)
    out_yps = nc.declare_dram_parameter("ypsum", [1, MM_N], fp32,
                                        isOutput=True)

    ytv = yt_d[:].rearrange("(n p f) -> n p f", p=P, f=f)
    ypv = yp_d[:].rearrange("(n p f) -> n p f", p=P, f=f)

    with ExitStack() as ctx:
        tc = ctx.enter_context(tile.TileContext(nc))
        accp = ctx.enter_context(tc.tile_pool(name="accp", bufs=1))
        psp = ctx.enter_context(tc.tile_pool(name="psum", bufs=1,
                                             space="PSUM"))

        acc_a = accp.tile([P, 2 * ntiles], fp32)
        scr = accp.tile([P, f], f16)
        ones = accp.tile([P, 1], f16)
        yps_sb = accp.tile([1, MM_N], fp32)
        g_sb = accp.tile([P, P], fp32)
        nc.vector.memset(ones, 1.0)
        ps = psp.tile([1, MM_N], fp32)
        gp = psp.tile([P, P], fp32)

        n_mm = f // MM_N
        n_gr = f // P

        loop = ExitStack()
        with loop:
            if repeat > 1:
                loop.enter_context(tc.For_i(0, repeat, 1))
            io = loop.enter_context(tc.tile_pool(name="io", bufs=io_bufs))
            mid = loop.enter_context(tc.tile_pool(name="mid", bufs=mid_bufs))

            yts, yps, rs, ms, r2s = {}, {}, {}, {}, {}

            for t in range(ntiles + s2):
                if t < ntiles:
                    yts[t] = io.tile([P, f], f16, tag="yt", name=f"yt{t}")
                    yps[t] = io.tile([P, f], f16, tag="yp", name=f"yp{t}")
                    nc.sync.dma_start(out=yts[t], in_=ytv[t])
                    nc.sync.dma_start(out=yps[t], in_=ypv[t])
                    yt, yp = yts[t], yps[t]
                    rs[t] = mid.tile([P, f], f16, tag="r", name=f"r{t}")
                    nc.vector.tensor_sub(rs[t], yt, yp)
                    ms[t] = mid.tile([P, f], f16, tag="m", name=f"m{t}")
                    if arch == "v2":
                        nc.vector.tensor_scalar(
                            ms[t], yt, 0.0, float(T_MID),
                            Alu.abs_max, Alu.is_le)
                    elif arch == "ts":
                        # |yt| on DVE (tensor_scalar abs_max, 4x-eligible)
                        nc.vector.tensor_scalar(
                            ms[t], yt, 0.0, None, Alu.abs_max)
                    elif arch == "bal":
                        # |yt| split across engines: fp16 abs is exact
                        # (sign bit) on both, so the mask is bit-identical
                        # either way. ACT is the busier engine (no 16-bit
                        # speedup + per-instr bubble), so give it only
                        # every other tile.
                        if t % 2 == 0:
                            nc.scalar.activation(ms[t], yt, Act.Abs)
                        else:
                            nc.vector.tensor_tensor(
                                ms[t], yt, yt, Alu.abs_max)
                    else:
                        nc.scalar.activation(ms[t], yt, Act.Abs)
                    for c in range(n_gr):
                        nc.tensor.matmul(
                            gp[:, :], yp[:, c * P:(c + 1) * P],
                            yp[:, c * P:(c + 1) * P],
                            start=(t == 0 and c == 0),
                            stop=(t == ntiles - 1 and c == n_gr - 1))
                    for c in range(n_mm):
                        nc.tensor.matmul(
                            ps[:, :], ones, yp[:, c * MM_N:(c + 1) * MM_N],
                            start=(t == 0 and c == 0),
                            stop=(t == ntiles - 1 and c == n_mm - 1))
                u = t - s1
                if 0 <= u < ntiles:
                    r2s[u] = mid.tile([P, f], f16, tag="r2", name=f"r2_{u}")
                    nc.scalar.activation(r2s[u], rs[u], Act.Square,
                                         accum_out=acc_a[:, u:u + 1])
                    del rs[u]
                u = t - s2
                if u >= 0:
                    col = acc_a[:, ntiles + u:ntiles + u + 1]
                    if arch == "v2":
                        nc.vector.tensor_tensor_reduce(
                            scr, r2s[u], ms[u], 1.0, 0.0,
                            Alu.mult, Alu.add, accum_out=col)
                    else:
                        nc.vector.scalar_tensor_tensor(
                            scr, ms[u], float(T_MID), r2s[u],
                            Alu.is_le, Alu.mult, accum_out=col)
                    del ms[u], r2s[u]

        nc.vector.tensor_copy(yps_sb, ps)
        nc.vector.tensor_copy(g_sb, gp)
        nc.sync.dma_start(out=out_acc[:, :], in_=acc_a)
        nc.sync.dma_start(out=out_g[:, :], in_=g_sb)
        nc.sync.dma_start(out=out_yps[:, :], in_=yps_sb)

    nc.finalize()
    _NC_CACHE[key] = nc
    return nc


# ---------------------------------------------------------------- device run
def make_in_maps(y_pred, y_true):
    """Cast to the streaming dtype and shard across cores."""
    yp16 = np.ascontiguousarray(y_pred, dtype=DTYPE)
    yt16 = np.ascontiguousarray(y_true, dtype=DTYPE)
    in_maps = []
    for i in range(NCORES):
        sl = slice(i * SHARD, (i + 1) * SHARD)
        in_maps.append({"y_true": yt16[sl], "y_pred": yp16[sl]})
    return in_maps


def run_device(y_pred, y_true, trace=False):
    _import_concourse()
    from concourse.bass_utils import run_bass_kernel_spmd

    nc = build_nc(**BEST_CFG)
    in_maps = make_in_maps(y_pred, y_true)
    return run_bass_kernel_spmd(nc, in_maps, list(range(NCORES)),
                                trace=trace)


def _combine(results):
    """Combine per-core device partials (float64)."""
    acc = np.stack([np.asarray(r["acc"], dtype=np.float64) for r in results])
    nt = acc.shape[-1] // 2
    s_r2 = acc[:, :, 0:nt].sum()
    s_mid = acc[:, :, nt:2 * nt].sum()
    s_yp2 = sum(
        np.diagonal(np.asarray(r["gram"], dtype=np.float64)).sum()
        for r in results
    )
    s_yp = np.stack([np.asarray(r["ypsum"], dtype=np.float64)
                     for r in results]).sum()
    return s_r2, s_yp2, s_mid, s_yp


# ------------------------------------------------------------- host finishing
def _f32_percentile_pos(n, pct):
    """Replicate jnp.percentile's float32 position arithmetic."""
    q = np.float32(np.float64(pct) / 100.0)
    nf = np.float32(n)
    pos = np.float32(q * np.float32(nf - np.float32(1.0)))
    low = np.floor(pos)
    high = np.ceil(pos)
    hw = np.float32(pos - low)
    lw = np.float32(np.float32(1.0) - hw)
    low = int(min(max(low, 0.0), float(n - 1)))
    high = int(min(max(high, 0.0), float(n - 1)))
    return low, high, lw, hw


def _fallback_numpy(y_pred, y_true):
    """Exact host computation (used only if the value band misses)."""
    y_pred = y_pred.astype(np.float32)
    y_true = y_true.astype(np.float32)
    n = y_true.size
    vs = np.sort(y_true)

    def pctl(pct):
        low, high, lw, hw = _f32_percentile_pos(n, pct)
        return np.float32(
            np.float32(vs[low] * lw) + np.float32(vs[high] * hw)
        )

    lo_t = pctl(LEFT_PCT)
    hi_t = pctl(RIGHT_PCT)
    r = (y_true - y_pred).astype(np.float32)
    r2 = (r * r).astype(np.float64)
    pen = np.where((y_true < lo_t) | (y_true > hi_t), PENALTY, 1.0)
    mse = (pen * r2).mean()
    var = y_pred.astype(np.float64).var(ddof=1)
    return np.float32(mse - VAR_W * var)


def _order_stat_threshold(win_sorted, base_rank, n, pct):
    """Exact percentile from a sorted value-band slice (or None)."""
    low, high, lw, hw = _f32_percentile_pos(n, pct)
    i_lo = low - base_rank
    i_hi = high - base_rank
    if (i_lo < 0 or i_hi < 0 or i_hi >= win_sorted.size
            or i_lo >= win_sorted.size):
        return None
    lv = win_sorted[i_lo]
    hv = win_sorted[i_hi]
    return np.float32(np.float32(lv * lw) + np.float32(hv * hw))


def kernel(y_pred, y_true):
    y_pred = np.asarray(y_pred, dtype=np.float32).reshape(-1)
    y_true = np.asarray(y_true, dtype=np.float32).reshape(-1)
    assert y_pred.shape == (N_TOTAL,) and y_true.shape == (N_TOTAL,)

    res = run_device(y_pred, y_true)
    s_r2, s_yp2, s_mid, s_yp = _combine(res.results)

    n = float(N_TOTAL)
    # exact global ranks of the band edges (host-side, integer-exact)
    c_l = int(np.count_nonzero(y_true < -T_OUT))
    c_r = int(np.count_nonzero(y_true > T_OUT))

    band_l = np.sort(y_true[(y_true >= -T_OUT) & (y_true <= -T_IN)])
    band_r = np.sort(y_true[(y_true >= T_IN) & (y_true <= T_OUT)])

    lo_t = _order_stat_threshold(band_l, c_l, N_TOTAL, LEFT_PCT)
    base_r = N_TOTAL - c_r - band_r.size
    hi_t = _order_stat_threshold(band_r, base_r, N_TOTAL, RIGHT_PCT)

    if (
        lo_t is None
        or hi_t is None
        or not (-float(T_OUT) < lo_t < -float(T_IN))
        or not (float(T_IN) < hi_t < float(T_OUT))
    ):
        return _fallback_numpy(y_pred, y_true)

    # exact correction over the bands: device penalized |fp16(y)| > T_MID,
    # we want y < lo_t or y > hi_t.  fp16 rounding moves values by
    # <= 2^-11 * 1.04 ~ 5e-4, so every disagreeing element lies inside
    # the (T_IN, T_OUT) bands.  Replay the device's fp16 arithmetic.
    sel = ((y_true >= -T_OUT) & (y_true <= -T_IN)) | (
        (y_true >= T_IN) & (y_true <= T_OUT)
    )
    yb = y_true[sel]
    yb16 = yb.astype(DTYPE)
    pb16 = y_pred[sel].astype(DTYPE)
    rb = (yb16.astype(np.float32) - pb16.astype(np.float32)).astype(DTYPE)
    r2b = (rb.astype(np.float32) ** 2).astype(DTYPE).astype(np.float64)
    want = (yb < lo_t) | (yb > hi_t)
    dev = np.abs(yb16.astype(np.float32)) > np.float32(T_MID)
    corr = (r2b * (want.astype(np.float64) - dev.astype(np.float64))).sum()

    tails = (s_r2 - s_mid) + corr
    mse = (s_r2 + (PENALTY - 1.0) * tails) / n
    var = (s_yp2 - (s_yp * s_yp) / n) / (n - 1.0)
    return np.float32(mse - VAR_W * var)


if __name__ == "__main__":
    rng = np.random.default_rng(0)
    yp = rng.standard_normal(N_TOTAL, dtype=np.float32)
    yt = rng.standard_normal(N_TOTAL, dtype=np.float32)
    print(kernel(yp, yt))
